# revision 46
# baseline (speedup 1.0000x reference)
"""HEPOS multi-head attention on 8 Trainium2 NeuronCores.

Sharding: 8 cores = 4 batches x 2 head-groups (8 heads each).  Since
stride == n_head, each key position feeds exactly one head, so K/V only
ever project through that head's 64 columns (16x less K/V-proj work than
the dense reference).  Per core: Q-proj -> per-head-pair K/V-proj ->
scores+exp -> PV (with an all-ones column block emitting the softmax
denominator for free) -> normalize -> out-proj.  Host sums the two
head-group partials per batch.

Performance structure (sim ~169us/core at 84% PE occupancy, vs ~201us
for the first working version whose HW time was ~615us):
  - all matmul operands bf16; every psum tile 512-wide (PSUM bank limit),
    rotating through two 4-deep one-bank pools (pA: Q-proj/scores/
    out-proj, pB: KV-proj/PV) so no producer ever waits on a consumer
  - both heads' score blocks are emitted before either PV block, so the
    Activation engine's exp (the attention-phase bottleneck at 1
    elem/cycle, dtype-independent) trails the PE without blocking it
  - qt stays in SBUF for the whole rep: letting its space be reused
    made the allocator gate the kg DMAs on all 128 Q-proj matmuls
  - input DMAs spread across the three DMA queues (SP / ACT / gpsimd
    SWDGE) and kg/vg prefetched one pair ahead: no single-queue
    serialization on the critical path
  - output partials stored bf16 (halves store + host-transfer bytes);
    host upcasts and combines in f32
  - Vg layout per (pair, l-chunk) is [h0 V | ones | h1 V] so one strided
    copy fills both heads and the ones block is shared
  - Q/K biases applied on-device; V bias folded into the host combine
    ((attn+bv)@Wo.T = attn@Wo.T + bv@Wo.T, a constant row); torch
    Linear y = x @ W.T + b matched throughout
"""

import numpy as np
from contextlib import ExitStack

import ml_dtypes

P = 128
BF16 = ml_dtypes.bfloat16

B, TQ_F, TK_F, D_F = 4, 2048, 8192, 1024
H_F, DH, STRIDE = 16, 64, 16
NCORES = 8
NH_LOC = H_F // 2
L_F = TK_F // STRIDE


def build_program(D=1024, TQ=2048, L=512, NH=8, num_devices=8, reps=1, loop_n=1):
    import concourse.bass as bass  # noqa: F401
    import concourse.tile as tile
    from concourse import bacc, mybir
    from concourse.masks import make_identity

    bf16 = mybir.dt.bfloat16
    f32 = mybir.dt.float32

    QD = NH * DH               # 512
    KD = D // P                # 8
    MQ = QD // P               # 4
    NP = NH // 2               # 4 head pairs
    LC = L // P                # 4
    T2 = TQ // 1024            # 2
    MT = TQ // P               # 16
    assert QD % P == 0 and D % P == 0 and L % P == 0 and TQ % 1024 == 0

    nc = bacc.Bacc(
        "TRN2",
        target_bir_lowering=False,
        debug=False,
        enable_asserts=False,
        num_devices=num_devices,
    )

    qt = nc.dram_tensor("qt", [D, TQ], bf16, kind="ExternalInput").ap()
    kgt = nc.dram_tensor("kgt", [D, NH, L], bf16, kind="ExternalInput").ap()
    vgt = nc.dram_tensor("vgt", [D, NH, L], bf16, kind="ExternalInput").ap()
    wqt = nc.dram_tensor("wqt", [D, QD], bf16, kind="ExternalInput").ap()
    wkt = nc.dram_tensor("wkt", [D, QD], bf16, kind="ExternalInput").ap()
    wvt = nc.dram_tensor("wvt", [D, QD], bf16, kind="ExternalInput").ap()
    wot = nc.dram_tensor("wot", [QD, D], bf16, kind="ExternalInput").ap()
    bq = nc.dram_tensor("bq", [P, MQ], f32, kind="ExternalInput").ap()
    bk = nc.dram_tensor("bk", [P, MQ], f32, kind="ExternalInput").ap()
    out = nc.dram_tensor("out", [TQ, D], bf16, kind="ExternalOutput").ap()

    qt_v = qt.rearrange("(kc p) t -> p kc t", p=P)
    kgt_v = kgt.rearrange("(kc p) h l -> p kc h l", p=P)
    vgt_v = vgt.rearrange("(kc p) h l -> p kc h l", p=P)
    wqt_v = wqt.rearrange("(kc p) m -> p kc m", p=P)
    wkt_v = wkt.rearrange("(kc p) m -> p kc m", p=P)
    wvt_v = wvt.rearrange("(kc p) m -> p kc m", p=P)
    wot_v = wot.rearrange("(j p) o -> p j o", p=P)
    out_v = out.rearrange("(mt p) o -> p mt o", p=P)

    Exp = mybir.ActivationFunctionType.Exp

    with tile.TileContext(nc) as tc, ExitStack() as octx:
        consts = octx.enter_context(tc.tile_pool(name="consts", bufs=1))
        persist = octx.enter_context(tc.tile_pool(name="persist", bufs=1))
        # pA: Q-proj + scores psums (512-wide, deep rotation so ACT exp can
        # trail PE without blocking it); pB: KV-proj + PV + out-proj psums.
        pA = octx.enter_context(tc.tile_pool(name="pA", bufs=4, space="PSUM"))
        pB = octx.enter_context(tc.tile_pool(name="pB", bufs=4, space="PSUM"))

        # Weights/biases go on the gpsimd (SWDGE) queue so the latency-critical
        # qt/kg/vg loads own the SP HWDGE queue; wq first (Q-proj needs it).
        ident = consts.tile([P, P], bf16)
        make_identity(nc, ident)
        # first m-chunk of wq on ACT (parallel with qt quarter-0 on SP) so
        # the first Q-proj psum group can start ~3.4us in; rest on gpsimd
        wq_sb = consts.tile([P, KD, QD], bf16)
        nc.scalar.dma_start(wq_sb[:, :, 0:P], wqt_v[:, :, 0:P])
        nc.gpsimd.dma_start(wq_sb[:, :, P:], wqt_v[:, :, P:])
        wk_sb = consts.tile([P, KD, QD], bf16)
        nc.gpsimd.dma_start(wk_sb[:], wkt_v)
        wv_sb = consts.tile([P, KD, QD], bf16)
        nc.gpsimd.dma_start(wv_sb[:], wvt_v)
        bq_sb = consts.tile([P, MQ], f32)
        nc.gpsimd.dma_start(bq_sb[:], bq)
        bk_sb = consts.tile([P, MQ], f32)
        nc.gpsimd.dma_start(bk_sb[:], bk)
        wo_sb = consts.tile([P, MQ, D], bf16)
        nc.gpsimd.dma_start(wo_sb[:], wot_v)
        bv_sb = None  # folded into host combine

        QT = persist.tile([P, MQ, TQ], bf16)
        KgT = persist.tile([P, NP, L], bf16)
        # Vg per (pair j, l-chunk): cols [h0 V | ones | h1 V] (3*DH wide).
        # lhsT for h0 = cols 0:128 (V,ones) -> psum[0:64]=out, [64:128]=denom;
        # lhsT for h1 = cols 64:192 (ones,V) -> psum[0:64]=denom, [64:128]=out.
        Vg = persist.tile([P, NP, LC, 3 * DH], bf16)
        nc.vector.memset(Vg[:, :, :, DH:2 * DH], 1.0)
        outT = persist.tile([P, MQ, TQ], bf16)

        if loop_n > 1:
            with tc.For_i(0, loop_n):
                _emit(nc, tc, mybir, bf16, f32, pA, pB,
                      qt_v, kgt_v, vgt_v, out_v,
                      wq_sb, wk_sb, wv_sb, wo_sb, bq_sb, bk_sb, bv_sb, ident,
                      QT, KgT, Vg, outT,
                      KD, MQ, NP, LC, T2, MT, DH, L, TQ, D, 0)
        else:
            for _rep in range(reps):
                _emit(nc, tc, mybir, bf16, f32, pA, pB,
                      qt_v, kgt_v, vgt_v, out_v,
                      wq_sb, wk_sb, wv_sb, wo_sb, bq_sb, bk_sb, bv_sb, ident,
                      QT, KgT, Vg, outT,
                      KD, MQ, NP, LC, T2, MT, DH, L, TQ, D, _rep)

    nc.compile()
    return nc


def _emit(nc, tc, mybir, bf16, f32, pA, pB,
          qt_v, kgt_v, vgt_v, out_v,
          wq_sb, wk_sb, wv_sb, wo_sb, bq_sb, bk_sb, bv_sb, ident,
          QT, KgT, Vg, outT,
          KD, MQ, NP, LC, T2, MT, DH, L, TQ, D, _rep):
    Exp = mybir.ActivationFunctionType.Exp
    with ExitStack() as ctx:
        # qt stays alive the whole rep: letting its SBUF be reused forced the
        # allocator to gate the kg DMAs on all 128 Q-proj matmuls draining.
        qt_pool = ctx.enter_context(tc.tile_pool(name=f"qtp{_rep}", bufs=1))
        qt_sb = qt_pool.tile([P, KD, TQ], bf16)
        # quarter 0 on SP (PE's first dependency; ACT pays the activation
        # table load first), quarter 1 on ACT in parallel.
        for tq in range(TQ // 512):
            eng = nc.scalar if tq == 1 else nc.sync
            eng.dma_start(
                qt_sb[:, :, tq * 512:(tq + 1) * 512],
                qt_v[:, :, tq * 512:(tq + 1) * 512],
            )
        for t2 in range(T2):
            for m in range(MQ):
                for th in range(2):
                    t0 = t2 * 1024 + th * 512
                    ps = pA.tile([P, 512], f32, tag="A")
                    for kc in range(KD):
                        nc.tensor.matmul(
                            ps[:],
                            wq_sb[:, kc, m * P:(m + 1) * P],
                            qt_sb[:, kc, t0:t0 + 512],
                            start=(kc == 0),
                            stop=(kc == KD - 1),
                        )
                    nc.vector.tensor_scalar_add(
                        QT[:, m, t0:t0 + 512], ps[:], bq_sb[:, m:m + 1]
                    )

        kg_pool = ctx.enter_context(tc.tile_pool(name=f"kg{_rep}", bufs=2))
        vg_pool = ctx.enter_context(tc.tile_pool(name=f"vg{_rep}", bufs=2))
        vt_pool = ctx.enter_context(tc.tile_pool(name=f"vt{_rep}", bufs=2))
        exp_pool = ctx.enter_context(tc.tile_pool(name=f"expp{_rep}", bufs=1))
        norm_pool = ctx.enter_context(tc.tile_pool(name=f"normp{_rep}", bufs=2))
        out_pool = ctx.enter_context(tc.tile_pool(name=f"outp{_rep}", bufs=2))

        def kv_dma(j):
            # Input streams spread over three DMA queues: kg-even on SP,
            # kg-odd on ACT (idle until the first exp), vg on gpsimd SWDGE.
            kg_sb = kg_pool.tile([P, KD, 2, L], bf16, tag="kg")
            kg_eng = nc.scalar if j == 1 else nc.sync
            kg_eng.dma_start(kg_sb[:], kgt_v[:, :, 2 * j:2 * j + 2, :])
            vg_in = vg_pool.tile([P, KD, 2, L], bf16, tag="vg")
            nc.gpsimd.dma_start(vg_in[:], vgt_v[:, :, 2 * j:2 * j + 2, :])
            return kg_sb, vg_in

        def kv_proj(j, kg_sb, vg_in):
            psk = pB.tile([P, 512], f32, tag="B")
            for hh in range(2):
                for kc in range(KD):
                    nc.tensor.matmul(
                        psk[hh * DH:(hh + 1) * DH, :L],
                        wk_sb[:, kc, j * P + hh * DH:j * P + (hh + 1) * DH],
                        kg_sb[:, kc, hh, :],
                        start=(kc == 0),
                        stop=(kc == KD - 1),
                        tile_position=(0, hh * DH),
                    )
            nc.vector.tensor_scalar_add(KgT[:, j, :], psk[:, :L], bk_sb[:, j:j + 1])

            psv = pB.tile([P, 512], f32, tag="B")
            for hh in range(2):
                for kc in range(KD):
                    nc.tensor.matmul(
                        psv[hh * DH:(hh + 1) * DH, :L],
                        wv_sb[:, kc, j * P + hh * DH:j * P + (hh + 1) * DH],
                        vg_in[:, kc, hh, :],
                        start=(kc == 0),
                        stop=(kc == KD - 1),
                        tile_position=(0, hh * DH),
                    )
            vgt_sb = vt_pool.tile([P, L], bf16, tag="vgt")
            nc.vector.tensor_copy(vgt_sb[:], psv[:, :L])
            for l in range(LC):
                pst = pB.tile([P, P], bf16, tag="B")
                nc.tensor.transpose(pst[:], vgt_sb[:, l * P:(l + 1) * P], ident)
                # h0 V -> cols 0:64, h1 V -> cols 128:192 in one strided copy
                dst = Vg[:, j, l, :].rearrange("p (a dh) -> p a dh", a=3)[:, 0::2, :]
                src = pst[:].rearrange("p (a dh) -> p a dh", a=2)
                nc.vector.tensor_copy(dst, src)

        def attention(j, t2):
            tsl = slice(t2 * 1024, (t2 + 1) * 1024)
            # scores+exp for BOTH heads first (512-wide, deep pss rotation),
            # then both PV blocks: ACT digests h0 exps while PE runs h1
            # scores, and PV never waits on a fresh activation.
            expst = [
                exp_pool.tile([P, LC, 1024], bf16, tag=f"expst{hh}",
                              name=f"expst{hh}")
                for hh in range(2)
            ]
            for hh in range(2):
                hsl = slice(hh * DH, (hh + 1) * DH)
                for l in range(LC):
                    for th in range(2):
                        t0 = t2 * 1024 + th * 512
                        pss = pA.tile([P, 512], f32, tag="A")
                        nc.tensor.matmul(
                            pss[:],
                            KgT[hsl, j, l * P:(l + 1) * P],
                            QT[hsl, j, t0:t0 + 512],
                            start=True,
                            stop=True,
                        )
                        nc.scalar.activation(
                            expst[hh][:, l, th * 512:(th + 1) * 512],
                            pss[:], Exp, scale=0.125,
                        )
            for hh in range(2):
                hsl = slice(hh * DH, (hh + 1) * DH)
                # h0: out rows 0:64, denom rows 64:128; h1: swapped
                osl = slice(0, DH) if hh == 0 else slice(DH, 2 * DH)
                dsl = slice(DH, 2 * DH) if hh == 0 else slice(0, DH)
                for th in range(2):
                    t0 = t2 * 1024 + th * 512
                    pspv = pB.tile([P, 512], f32, tag="B")
                    for l in range(LC):
                        nc.tensor.matmul(
                            pspv[:],
                            Vg[:, j, l, hh * DH:hh * DH + 2 * DH],
                            expst[hh][:, l, th * 512:(th + 1) * 512],
                            start=(l == 0),
                            stop=(l == LC - 1),
                        )
                    recip = norm_pool.tile([DH, 512], f32, tag="recip")
                    nc.vector.reciprocal(recip[:], pspv[dsl, :])
                    nc.vector.tensor_mul(
                        outT[hsl, j, t0:t0 + 512], pspv[osl, :], recip[:]
                    )
            # V bias is folded into the host combine: (attn+bv)@WoT =
            # attn@WoT + bv@WoT, and bv@WoT is a constant row added with bo.

        def out_proj(mts, pool=None, ptag="A"):
            # psums in pA (scores pool, idle during out-proj): decouples the
            # out-proj from pB's pspv rotation, whose readers are the DVE
            # normalize chain.
            pool = pool or pA
            for mt in mts:
                ob = out_pool.tile([P, D], bf16, tag="ob")
                for th in range(2):
                    pso = pool.tile([P, 512], f32, tag=ptag, name="pso")
                    for jo in range(MQ):
                        nc.tensor.matmul(
                            pso[:],
                            outT[:, jo, mt * P:(mt + 1) * P],
                            wo_sb[:, jo, th * 512:(th + 1) * 512],
                            start=(jo == 0),
                            stop=(jo == MQ - 1),
                        )
                    nc.vector.tensor_copy(ob[:, th * 512:(th + 1) * 512], pso[:])
                out_eng = nc.gpsimd if mt % 2 == 0 else nc.sync
                out_eng.dma_start(out_v[:, mt, :], ob[:])

        # kv DMAs prefetched one pair ahead (bufs=2) so transfer of pair j+1
        # overlaps compute of pair j on the single SP HWDGE queue.
        pending = {0: kv_dma(0), 1: kv_dma(1)}
        for j in range(NP):
            kg_sb, vg_in = pending.pop(j)
            kv_proj(j, kg_sb, vg_in)
            attention(j, 0)
            if j + 2 < NP:
                pending[j + 2] = kv_dma(j + 2)
        out_proj(range(0, MT // T2))
        for j in range(NP):
            attention(j, 1)
        out_proj(range(MT // T2, MT))


_PROG = None


def _get_program():
    global _PROG
    if _PROG is None:
        _PROG = build_program(D=D_F, TQ=TQ_F, L=L_F, NH=NH_LOC, num_devices=NCORES)
    return _PROG


def make_core_inputs(q, k, v, Wq, bq, Wk, bk, Wv, bv, Wo, bo):
    D, L, S, NH = D_F, L_F, STRIDE, NH_LOC
    QD = NH * DH
    MQ = QD // P

    q = np.ascontiguousarray(np.asarray(q, np.float32))
    k = np.ascontiguousarray(np.asarray(k, np.float32))
    v = np.ascontiguousarray(np.asarray(v, np.float32))

    qT = [np.ascontiguousarray(q[b].T).astype(BF16) for b in range(B)]
    kr = [k[b].reshape(L, S, D) for b in range(B)]
    vr = [v[b].reshape(L, S, D) for b in range(B)]

    WqT = np.ascontiguousarray(np.asarray(Wq, np.float32).T).astype(BF16)
    WkT = np.ascontiguousarray(np.asarray(Wk, np.float32).T).astype(BF16)
    WvT = np.ascontiguousarray(np.asarray(Wv, np.float32).T).astype(BF16)
    WoT = np.ascontiguousarray(np.asarray(Wo, np.float32).T).astype(BF16)
    bq = np.asarray(bq, np.float32)
    bk = np.asarray(bk, np.float32)
    bv = np.asarray(bv, np.float32)

    in_maps = []
    for c in range(NCORES):
        b, g = divmod(c, 2)
        gsl = slice(g * QD, (g + 1) * QD)
        hs0 = g * NH
        kgt = np.ascontiguousarray(
            kr[b][:, hs0:hs0 + NH, :].transpose(2, 1, 0)
        ).astype(BF16)
        vgt = np.ascontiguousarray(
            vr[b][:, hs0:hs0 + NH, :].transpose(2, 1, 0)
        ).astype(BF16)
        in_maps.append(
            {
                "qt": qT[b],
                "kgt": kgt,
                "vgt": vgt,
                "wqt": np.ascontiguousarray(WqT[:, gsl]),
                "wkt": np.ascontiguousarray(WkT[:, gsl]),
                "wvt": np.ascontiguousarray(WvT[:, gsl]),
                "wot": np.ascontiguousarray(WoT[gsl, :]),
                "bq": np.ascontiguousarray(bq[gsl].reshape(MQ, P).T),
                "bk": np.ascontiguousarray(bk[gsl].reshape(MQ, P).T),
            }
        )
    return in_maps


def combine_outputs(results, bo, bv, Wo):
    # device partials exclude the V bias: (attn+bv)@Wo.T = attn@Wo.T + bv@Wo.T,
    # so add bv@Wo.T (a constant row) here along with bo.
    bo = np.asarray(bo, np.float32)
    bvWo = np.asarray(bv, np.float32) @ np.asarray(Wo, np.float32).T
    out = np.empty((B, TQ_F, D_F), np.float32)
    for b in range(B):
        out[b] = (
            results[2 * b]["out"].astype(np.float32)
            + results[2 * b + 1]["out"].astype(np.float32)
            + (bo + bvWo)
        )
    return out


def kernel(q, k, v, Wq, bq, Wk, bk, Wv, bv, Wo, bo):
    from concourse.bass_utils import run_bass_kernel_spmd

    nc = _get_program()
    in_maps = make_core_inputs(q, k, v, Wq, bq, Wk, bk, Wv, bv, Wo, bo)
    res = run_bass_kernel_spmd(nc, in_maps, core_ids=list(range(NCORES)))
    return combine_outputs(res.results, bo, bv, Wo)


# revision 60
# speedup vs baseline: 1.0515x; 1.0515x over previous
"""HEPOS multi-head attention on 8 Trainium2 NeuronCores.

Sharding: 8 cores = 4 batches x 2 head-groups (8 heads each).  Since
stride == n_head, each key position feeds exactly one head, so K/V only
ever project through that head's 64 columns (16x less K/V-proj work than
the dense reference).  Per core: Q-proj -> per-head-pair K/V-proj ->
scores+exp -> PV (with an all-ones column block emitting the softmax
denominator for free) -> normalize -> out-proj.  Host sums the two
head-group partials per batch.

Performance structure (sim ~169us/core at 84% PE occupancy, vs ~201us
for the first working version whose HW time was ~615us):
  - all matmul operands bf16; every psum tile 512-wide (PSUM bank limit),
    rotating through two 4-deep one-bank pools (pA: Q-proj/scores/
    out-proj, pB: KV-proj/PV) so no producer ever waits on a consumer
  - both heads' score blocks are emitted before either PV block, so the
    Activation engine's exp (the attention-phase bottleneck at 1
    elem/cycle, dtype-independent) trails the PE without blocking it
  - qt stays in SBUF for the whole rep: letting its space be reused
    made the allocator gate the kg DMAs on all 128 Q-proj matmuls
  - input DMAs spread across the three DMA queues (SP / ACT / gpsimd
    SWDGE) and kg/vg prefetched one pair ahead: no single-queue
    serialization on the critical path
  - output partials stored bf16 (halves store + host-transfer bytes);
    host upcasts and combines in f32
  - Vg layout per (pair, l-chunk) is [h0 V | ones | h1 V] so one strided
    copy fills both heads and the ones block is shared
  - Q/K biases applied on-device; V bias folded into the host combine
    ((attn+bv)@Wo.T = attn@Wo.T + bv@Wo.T, a constant row); torch
    Linear y = x @ W.T + b matched throughout
"""

import numpy as np
from contextlib import ExitStack

import ml_dtypes

P = 128
BF16 = ml_dtypes.bfloat16

B, TQ_F, TK_F, D_F = 4, 2048, 8192, 1024
H_F, DH, STRIDE = 16, 64, 16
NCORES = 8
NH_LOC = H_F // 2
L_F = TK_F // STRIDE


def build_program(D=1024, TQ=2048, L=512, NH=8, num_devices=8, reps=1, loop_n=1):
    import concourse.bass as bass  # noqa: F401
    import concourse.tile as tile
    from concourse import bacc, mybir
    from concourse.masks import make_identity

    bf16 = mybir.dt.bfloat16
    f32 = mybir.dt.float32

    QD = NH * DH               # 512
    KD = D // P                # 8
    MQ = QD // P               # 4
    NP = NH // 2               # 4 head pairs
    LC = L // P                # 4
    T2 = TQ // 1024            # 2
    MT = TQ // P               # 16
    assert QD % P == 0 and D % P == 0 and L % P == 0 and TQ % 1024 == 0

    nc = bacc.Bacc(
        "TRN2",
        target_bir_lowering=False,
        debug=False,
        enable_asserts=False,
        num_devices=num_devices,
    )

    qt = nc.dram_tensor("qt", [D, TQ], bf16, kind="ExternalInput").ap()
    kgt = nc.dram_tensor("kgt", [D, NH, L], bf16, kind="ExternalInput").ap()
    vgt = nc.dram_tensor("vgt", [D, NH, L], bf16, kind="ExternalInput").ap()
    wqt = nc.dram_tensor("wqt", [D, QD], bf16, kind="ExternalInput").ap()
    wkt = nc.dram_tensor("wkt", [D, QD], bf16, kind="ExternalInput").ap()
    wvt = nc.dram_tensor("wvt", [D, QD], bf16, kind="ExternalInput").ap()
    wot = nc.dram_tensor("wot", [QD, D], bf16, kind="ExternalInput").ap()
    bq = nc.dram_tensor("bq", [P, MQ], f32, kind="ExternalInput").ap()
    bk = nc.dram_tensor("bk", [P, MQ], f32, kind="ExternalInput").ap()
    out = nc.dram_tensor("out", [TQ, D], bf16, kind="ExternalOutput").ap()

    qt_v = qt.rearrange("(kc p) t -> p kc t", p=P)
    kgt_v = kgt.rearrange("(kc p) h l -> p kc h l", p=P)
    vgt_v = vgt.rearrange("(kc p) h l -> p kc h l", p=P)
    wqt_v = wqt.rearrange("(kc p) m -> p kc m", p=P)
    wkt_v = wkt.rearrange("(kc p) m -> p kc m", p=P)
    wvt_v = wvt.rearrange("(kc p) m -> p kc m", p=P)
    wot_v = wot.rearrange("(j p) o -> p j o", p=P)
    out_v = out.rearrange("(mt p) o -> p mt o", p=P)

    Exp = mybir.ActivationFunctionType.Exp

    with tile.TileContext(nc) as tc, ExitStack() as octx:
        consts = octx.enter_context(tc.tile_pool(name="consts", bufs=1))
        persist = octx.enter_context(tc.tile_pool(name="persist", bufs=1))
        # pA: Q-proj + scores psums (512-wide, deep rotation so ACT exp can
        # trail PE without blocking it); pB: KV-proj + PV + out-proj psums.
        pA = octx.enter_context(tc.tile_pool(name="pA", bufs=4, space="PSUM"))
        pB = octx.enter_context(tc.tile_pool(name="pB", bufs=4, space="PSUM"))

        # Weights/biases go on the gpsimd (SWDGE) queue so the latency-critical
        # qt/kg/vg loads own the SP HWDGE queue; wq first (Q-proj needs it).
        ident = consts.tile([P, P], bf16)
        make_identity(nc, ident)
        # first m-chunk of wq on ACT (parallel with qt quarter-0 on SP) so
        # the first Q-proj psum group can start ~3.4us in; rest on gpsimd
        wq_sb = consts.tile([P, KD, QD], bf16)
        nc.scalar.dma_start(wq_sb[:, :, 0:P], wqt_v[:, :, 0:P])
        nc.gpsimd.dma_start(wq_sb[:, :, P:], wqt_v[:, :, P:])
        wk_sb = consts.tile([P, KD, QD], bf16)
        nc.gpsimd.dma_start(wk_sb[:], wkt_v)
        wv_sb = consts.tile([P, KD, QD], bf16)
        nc.gpsimd.dma_start(wv_sb[:], wvt_v)
        bq_sb = consts.tile([P, MQ], f32)
        nc.gpsimd.dma_start(bq_sb[:], bq)
        bk_sb = consts.tile([P, MQ], f32)
        nc.gpsimd.dma_start(bk_sb[:], bk)
        wo_sb = consts.tile([P, MQ, D], bf16)
        nc.gpsimd.dma_start(wo_sb[:], wot_v)
        bv_sb = None  # folded into host combine

        QT = persist.tile([P, MQ, TQ], bf16)
        KgT = persist.tile([P, NP, L], bf16)
        # Vg per (pair j, l-chunk): cols [h0 V | ones | h1 V] (3*DH wide).
        # lhsT for h0 = cols 0:128 (V,ones) -> psum[0:64]=out, [64:128]=denom;
        # lhsT for h1 = cols 64:192 (ones,V) -> psum[0:64]=denom, [64:128]=out.
        Vg = persist.tile([P, NP, LC, 3 * DH], bf16)
        nc.vector.memset(Vg[:, :, :, DH:2 * DH], 1.0)
        outT = persist.tile([P, MQ, TQ], bf16)

        if loop_n > 1:
            with tc.For_i(0, loop_n):
                _emit(nc, tc, mybir, bf16, f32, pA, pB,
                      qt_v, kgt_v, vgt_v, out_v,
                      wq_sb, wk_sb, wv_sb, wo_sb, bq_sb, bk_sb, bv_sb, ident,
                      QT, KgT, Vg, outT,
                      KD, MQ, NP, LC, T2, MT, DH, L, TQ, D, 0)
        else:
            for _rep in range(reps):
                _emit(nc, tc, mybir, bf16, f32, pA, pB,
                      qt_v, kgt_v, vgt_v, out_v,
                      wq_sb, wk_sb, wv_sb, wo_sb, bq_sb, bk_sb, bv_sb, ident,
                      QT, KgT, Vg, outT,
                      KD, MQ, NP, LC, T2, MT, DH, L, TQ, D, _rep)

    nc.compile()
    return nc


def _emit(nc, tc, mybir, bf16, f32, pA, pB,
          qt_v, kgt_v, vgt_v, out_v,
          wq_sb, wk_sb, wv_sb, wo_sb, bq_sb, bk_sb, bv_sb, ident,
          QT, KgT, Vg, outT,
          KD, MQ, NP, LC, T2, MT, DH, L, TQ, D, _rep):
    Exp = mybir.ActivationFunctionType.Exp
    with ExitStack() as ctx:
        # qt stays alive the whole rep: letting its SBUF be reused forced the
        # allocator to gate the kg DMAs on all 128 Q-proj matmuls draining.
        qt_pool = ctx.enter_context(tc.tile_pool(name=f"qtp{_rep}", bufs=1))
        qt_sb = qt_pool.tile([P, KD, TQ], bf16)
        # quarter 0 on SP (PE's first dependency; ACT pays the activation
        # table load first), quarter 1 on ACT in parallel.
        for tq in range(TQ // 512):
            eng = nc.scalar if tq == 1 else nc.sync
            eng.dma_start(
                qt_sb[:, :, tq * 512:(tq + 1) * 512],
                qt_v[:, :, tq * 512:(tq + 1) * 512],
            )
        for t2 in range(T2):
            for m in range(MQ):
                for th in range(2):
                    t0 = t2 * 1024 + th * 512
                    ps = pA.tile([P, 512], f32, tag="A")
                    for kc in range(KD):
                        nc.tensor.matmul(
                            ps[:],
                            wq_sb[:, kc, m * P:(m + 1) * P],
                            qt_sb[:, kc, t0:t0 + 512],
                            start=(kc == 0),
                            stop=(kc == KD - 1),
                        )
                    nc.vector.tensor_scalar_add(
                        QT[:, m, t0:t0 + 512], ps[:], bq_sb[:, m:m + 1]
                    )

        kg_pool = ctx.enter_context(tc.tile_pool(name=f"kg{_rep}", bufs=2))
        vg_pool = ctx.enter_context(tc.tile_pool(name=f"vg{_rep}", bufs=2))
        vt_pool = ctx.enter_context(tc.tile_pool(name=f"vt{_rep}", bufs=2))
        exp_pool = ctx.enter_context(tc.tile_pool(name=f"expp{_rep}", bufs=1))
        norm_pool = ctx.enter_context(tc.tile_pool(name=f"normp{_rep}", bufs=2))
        out_pool = ctx.enter_context(tc.tile_pool(name=f"outp{_rep}", bufs=2))

        def kv_dma(j):
            # Input streams spread over three DMA queues: kg-even on SP,
            # kg-odd on ACT (idle until the first exp), vg on gpsimd SWDGE.
            kg_sb = kg_pool.tile([P, KD, 2, L], bf16, tag="kg")
            kg_eng = nc.scalar if j == 1 else nc.sync
            kg_eng.dma_start(kg_sb[:], kgt_v[:, :, 2 * j:2 * j + 2, :])
            vg_in = vg_pool.tile([P, KD, 2, L], bf16, tag="vg")
            nc.gpsimd.dma_start(vg_in[:], vgt_v[:, :, 2 * j:2 * j + 2, :])
            return kg_sb, vg_in

        def kv_proj(j, kg_sb, vg_in):
            psk = pB.tile([P, 512], f32, tag="B")
            for hh in range(2):
                for kc in range(KD):
                    nc.tensor.matmul(
                        psk[hh * DH:(hh + 1) * DH, :L],
                        wk_sb[:, kc, j * P + hh * DH:j * P + (hh + 1) * DH],
                        kg_sb[:, kc, hh, :],
                        start=(kc == 0),
                        stop=(kc == KD - 1),
                        tile_position=(0, hh * DH),
                    )
            nc.vector.tensor_scalar_add(KgT[:, j, :], psk[:, :L], bk_sb[:, j:j + 1])

            psv = pB.tile([P, 512], f32, tag="B")
            for hh in range(2):
                for kc in range(KD):
                    nc.tensor.matmul(
                        psv[hh * DH:(hh + 1) * DH, :L],
                        wv_sb[:, kc, j * P + hh * DH:j * P + (hh + 1) * DH],
                        vg_in[:, kc, hh, :],
                        start=(kc == 0),
                        stop=(kc == KD - 1),
                        tile_position=(0, hh * DH),
                    )
            vgt_sb = vt_pool.tile([P, L], bf16, tag="vgt")
            nc.vector.tensor_copy(vgt_sb[:], psv[:, :L])
            for l in range(LC):
                pst = pB.tile([P, P], bf16, tag="B")
                nc.tensor.transpose(pst[:], vgt_sb[:, l * P:(l + 1) * P], ident)
                # h0 V -> cols 0:64, h1 V -> cols 128:192 in one strided copy
                dst = Vg[:, j, l, :].rearrange("p (a dh) -> p a dh", a=3)[:, 0::2, :]
                src = pst[:].rearrange("p (a dh) -> p a dh", a=2)
                nc.vector.tensor_copy(dst, src)

        def scores_part(j, t2):
            # all 16 score+exp tiles emit before any PV: each PV block then
            # gates on exps finished 2-3 blocks ago, so the ACT deficit
            # (exp ~9.8us/att vs PE 6.8us) hides under PE's own score work
            expst = {
                (hh, th): exp_pool.tile([P, LC, 512], bf16,
                                        tag=f"e{hh}{th}", name=f"e{hh}{th}")
                for hh in range(2) for th in range(2)
            }
            for hh in range(2):
                hsl = slice(hh * DH, (hh + 1) * DH)
                for th in range(2):
                    t0 = t2 * 1024 + th * 512
                    for l in range(LC):
                        pss = pA.tile([P, 512], f32, tag="A")
                        nc.tensor.matmul(
                            pss[:],
                            KgT[hsl, j, l * P:(l + 1) * P],
                            QT[hsl, j, t0:t0 + 512],
                            start=True,
                            stop=True,
                        )
                        nc.scalar.activation(
                            expst[hh, th][:, l, :], pss[:], Exp, scale=0.125,
                        )
            return expst

        def pv_part(j, t2, expst):
            for hh in range(2):
                hsl = slice(hh * DH, (hh + 1) * DH)
                # h0: out rows 0:64, denom rows 64:128; h1: swapped
                osl = slice(0, DH) if hh == 0 else slice(DH, 2 * DH)
                dsl = slice(DH, 2 * DH) if hh == 0 else slice(0, DH)
                for th in range(2):
                    t0 = t2 * 1024 + th * 512
                    pspv = pB.tile([P, 512], f32, tag="B")
                    for l in range(LC):
                        nc.tensor.matmul(
                            pspv[:],
                            Vg[:, j, l, hh * DH:hh * DH + 2 * DH],
                            expst[hh, th][:, l, :],
                            start=(l == 0),
                            stop=(l == LC - 1),
                        )
                    recip = norm_pool.tile([DH, 512], f32, tag="recip")
                    nc.vector.reciprocal(recip[:], pspv[dsl, :])
                    nc.vector.tensor_mul(
                        outT[hsl, j, t0:t0 + 512], pspv[osl, :], recip[:]
                    )
            # V bias is folded into the host combine: (attn+bv)@WoT =
            # attn@WoT + bv@WoT, and bv@WoT is a constant row added with bo.

        def out_proj(mts, pool=None, ptag="A"):
            # psums in pA (scores pool, idle during out-proj): decouples the
            # out-proj from pB's pspv rotation, whose readers are the DVE
            # normalize chain.
            pool = pool or pA
            for mt in mts:
                ob = out_pool.tile([P, D], bf16, tag="ob")
                for th in range(2):
                    pso = pool.tile([P, 512], f32, tag=ptag, name="pso")
                    for jo in range(MQ):
                        nc.tensor.matmul(
                            pso[:],
                            outT[:, jo, mt * P:(mt + 1) * P],
                            wo_sb[:, jo, th * 512:(th + 1) * 512],
                            start=(jo == 0),
                            stop=(jo == MQ - 1),
                        )
                    nc.vector.tensor_copy(ob[:, th * 512:(th + 1) * 512], pso[:])
                out_eng = nc.gpsimd if mt % 2 == 0 else nc.sync
                out_eng.dma_start(out_v[:, mt, :], ob[:])

        # kv DMAs prefetched one pair ahead (bufs=2) so transfer of pair j+1
        # overlaps compute of pair j on the single SP HWDGE queue.  Software
        # pipeline: kv_proj(j+1)'s exp-free PE work sits between scores(j)
        # and PV(j) so the Activation engine finishes j's exps in time.
        pending = {0: kv_dma(0), 1: kv_dma(1)}
        kv_proj(0, *pending.pop(0))
        for j in range(NP):
            exp_new = scores_part(j, 0)
            if j + 2 < NP:
                pending[j + 2] = kv_dma(j + 2)
            if j + 1 < NP:
                kv_proj(j + 1, *pending.pop(j + 1))
            pv_part(j, 0, exp_new)
        # t2=0 out-proj chunks play the same role for the t2=1 blocks.
        HM = MT // T2 // NP
        for j in range(NP):
            expst = scores_part(j, 1)
            out_proj(range(j * HM, (j + 1) * HM))
            pv_part(j, 1, expst)
        out_proj(range(MT // T2, MT))


_PROG = None


def _get_program():
    global _PROG
    if _PROG is None:
        _PROG = build_program(D=D_F, TQ=TQ_F, L=L_F, NH=NH_LOC, num_devices=NCORES)
    return _PROG


def make_core_inputs(q, k, v, Wq, bq, Wk, bk, Wv, bv, Wo, bo):
    D, L, S, NH = D_F, L_F, STRIDE, NH_LOC
    QD = NH * DH
    MQ = QD // P

    q = np.ascontiguousarray(np.asarray(q, np.float32))
    k = np.ascontiguousarray(np.asarray(k, np.float32))
    v = np.ascontiguousarray(np.asarray(v, np.float32))

    qT = [np.ascontiguousarray(q[b].T).astype(BF16) for b in range(B)]
    kr = [k[b].reshape(L, S, D) for b in range(B)]
    vr = [v[b].reshape(L, S, D) for b in range(B)]

    WqT = np.ascontiguousarray(np.asarray(Wq, np.float32).T).astype(BF16)
    WkT = np.ascontiguousarray(np.asarray(Wk, np.float32).T).astype(BF16)
    WvT = np.ascontiguousarray(np.asarray(Wv, np.float32).T).astype(BF16)
    WoT = np.ascontiguousarray(np.asarray(Wo, np.float32).T).astype(BF16)
    bq = np.asarray(bq, np.float32)
    bk = np.asarray(bk, np.float32)
    bv = np.asarray(bv, np.float32)

    in_maps = []
    for c in range(NCORES):
        b, g = divmod(c, 2)
        gsl = slice(g * QD, (g + 1) * QD)
        hs0 = g * NH
        kgt = np.ascontiguousarray(
            kr[b][:, hs0:hs0 + NH, :].transpose(2, 1, 0)
        ).astype(BF16)
        vgt = np.ascontiguousarray(
            vr[b][:, hs0:hs0 + NH, :].transpose(2, 1, 0)
        ).astype(BF16)
        in_maps.append(
            {
                "qt": qT[b],
                "kgt": kgt,
                "vgt": vgt,
                "wqt": np.ascontiguousarray(WqT[:, gsl]),
                "wkt": np.ascontiguousarray(WkT[:, gsl]),
                "wvt": np.ascontiguousarray(WvT[:, gsl]),
                "wot": np.ascontiguousarray(WoT[gsl, :]),
                "bq": np.ascontiguousarray(bq[gsl].reshape(MQ, P).T),
                "bk": np.ascontiguousarray(bk[gsl].reshape(MQ, P).T),
            }
        )
    return in_maps


def combine_outputs(results, bo, bv, Wo):
    # device partials exclude the V bias: (attn+bv)@Wo.T = attn@Wo.T + bv@Wo.T,
    # so add bv@Wo.T (a constant row) here along with bo.
    bo = np.asarray(bo, np.float32)
    bvWo = np.asarray(bv, np.float32) @ np.asarray(Wo, np.float32).T
    out = np.empty((B, TQ_F, D_F), np.float32)
    for b in range(B):
        out[b] = (
            results[2 * b]["out"].astype(np.float32)
            + results[2 * b + 1]["out"].astype(np.float32)
            + (bo + bvWo)
        )
    return out


def kernel(q, k, v, Wq, bq, Wk, bk, Wv, bv, Wo, bo):
    from concourse.bass_utils import run_bass_kernel_spmd

    nc = _get_program()
    in_maps = make_core_inputs(q, k, v, Wq, bq, Wk, bk, Wv, bv, Wo, bo)
    res = run_bass_kernel_spmd(nc, in_maps, core_ids=list(range(NCORES)))
    return combine_outputs(res.results, bo, bv, Wo)


# revision 61
# speedup vs baseline: 1.0967x; 1.0430x over previous
"""HEPOS multi-head attention on 8 Trainium2 NeuronCores.

Sharding: 8 cores = 4 batches x 2 head-groups (8 heads each).  Since
stride == n_head, each key position feeds exactly one head, so K/V only
ever project through that head's 64 columns (16x less K/V-proj work than
the dense reference).  Per core: Q-proj -> per-head-pair K/V-proj ->
scores+exp -> PV (with an all-ones column block emitting the softmax
denominator for free) -> normalize -> out-proj.  Host sums the two
head-group partials per batch.

Performance structure (sim ~160us/core at 87% PE occupancy, vs ~201us
for the first working version whose HW time was ~615us):
  - software-pipelined attention: scores(j)+exp emit, then exp-free PE
    work (kv-proj of pair j+1 at t2=0, out-proj chunks at t2=1), then
    PV(j) — the Activation engine's exp deficit (9.8us/att vs PE 6.8us)
    hides under PE work instead of stalling it
  - all matmul operands bf16; every psum tile 512-wide (PSUM bank limit),
    rotating through two 4-deep one-bank pools (pA: Q-proj/scores/
    out-proj, pB: KV-proj/PV) so no producer ever waits on a consumer
  - both heads' score blocks are emitted before either PV block, so the
    Activation engine's exp (the attention-phase bottleneck at 1
    elem/cycle, dtype-independent) trails the PE without blocking it
  - qt stays in SBUF for the whole rep: letting its space be reused
    made the allocator gate the kg DMAs on all 128 Q-proj matmuls
  - input DMAs spread across the three DMA queues (SP / ACT / gpsimd
    SWDGE) and kg/vg prefetched one pair ahead: no single-queue
    serialization on the critical path
  - output partials stored bf16 (halves store + host-transfer bytes);
    host upcasts and combines in f32
  - Vg layout per (pair, l-chunk) is [h0 V | ones | h1 V] so one strided
    copy fills both heads and the ones block is shared
  - Q/K biases applied on-device; V bias folded into the host combine
    ((attn+bv)@Wo.T = attn@Wo.T + bv@Wo.T, a constant row); torch
    Linear y = x @ W.T + b matched throughout
"""

import numpy as np
from contextlib import ExitStack

import ml_dtypes

P = 128
BF16 = ml_dtypes.bfloat16

B, TQ_F, TK_F, D_F = 4, 2048, 8192, 1024
H_F, DH, STRIDE = 16, 64, 16
NCORES = 8
NH_LOC = H_F // 2
L_F = TK_F // STRIDE


def build_program(D=1024, TQ=2048, L=512, NH=8, num_devices=8, reps=1, loop_n=1):
    import concourse.bass as bass  # noqa: F401
    import concourse.tile as tile
    from concourse import bacc, mybir
    from concourse.masks import make_identity

    bf16 = mybir.dt.bfloat16
    f32 = mybir.dt.float32

    QD = NH * DH               # 512
    KD = D // P                # 8
    MQ = QD // P               # 4
    NP = NH // 2               # 4 head pairs
    LC = L // P                # 4
    T2 = TQ // 1024            # 2
    MT = TQ // P               # 16
    assert QD % P == 0 and D % P == 0 and L % P == 0 and TQ % 1024 == 0

    nc = bacc.Bacc(
        "TRN2",
        target_bir_lowering=False,
        debug=False,
        enable_asserts=False,
        num_devices=num_devices,
    )

    qt = nc.dram_tensor("qt", [D, TQ], bf16, kind="ExternalInput").ap()
    kgt = nc.dram_tensor("kgt", [D, NH, L], bf16, kind="ExternalInput").ap()
    vgt = nc.dram_tensor("vgt", [D, NH, L], bf16, kind="ExternalInput").ap()
    wqt = nc.dram_tensor("wqt", [D, QD], bf16, kind="ExternalInput").ap()
    wkt = nc.dram_tensor("wkt", [D, QD], bf16, kind="ExternalInput").ap()
    wvt = nc.dram_tensor("wvt", [D, QD], bf16, kind="ExternalInput").ap()
    wot = nc.dram_tensor("wot", [QD, D], bf16, kind="ExternalInput").ap()
    bq = nc.dram_tensor("bq", [P, MQ], f32, kind="ExternalInput").ap()
    bk = nc.dram_tensor("bk", [P, MQ], f32, kind="ExternalInput").ap()
    out = nc.dram_tensor("out", [TQ, D], bf16, kind="ExternalOutput").ap()

    qt_v = qt.rearrange("(kc p) t -> p kc t", p=P)
    kgt_v = kgt.rearrange("(kc p) h l -> p kc h l", p=P)
    vgt_v = vgt.rearrange("(kc p) h l -> p kc h l", p=P)
    wqt_v = wqt.rearrange("(kc p) m -> p kc m", p=P)
    wkt_v = wkt.rearrange("(kc p) m -> p kc m", p=P)
    wvt_v = wvt.rearrange("(kc p) m -> p kc m", p=P)
    wot_v = wot.rearrange("(j p) o -> p j o", p=P)
    out_v = out.rearrange("(mt p) o -> p mt o", p=P)

    Exp = mybir.ActivationFunctionType.Exp

    with tile.TileContext(nc) as tc, ExitStack() as octx:
        consts = octx.enter_context(tc.tile_pool(name="consts", bufs=1))
        persist = octx.enter_context(tc.tile_pool(name="persist", bufs=1))
        # pA: Q-proj + scores psums (512-wide, deep rotation so ACT exp can
        # trail PE without blocking it); pB: KV-proj + PV + out-proj psums.
        pA = octx.enter_context(tc.tile_pool(name="pA", bufs=4, space="PSUM"))
        pB = octx.enter_context(tc.tile_pool(name="pB", bufs=4, space="PSUM"))

        # Weights/biases go on the gpsimd (SWDGE) queue so the latency-critical
        # qt/kg/vg loads own the SP HWDGE queue; wq first (Q-proj needs it).
        ident = consts.tile([P, P], bf16)
        make_identity(nc, ident)
        # first m-chunk of wq on ACT (parallel with qt quarter-0 on SP) so
        # the first Q-proj psum group can start ~3.4us in; rest on gpsimd
        wq_sb = consts.tile([P, KD, QD], bf16)
        nc.scalar.dma_start(wq_sb[:, :, 0:P], wqt_v[:, :, 0:P])
        nc.gpsimd.dma_start(wq_sb[:, :, P:], wqt_v[:, :, P:])
        wk_sb = consts.tile([P, KD, QD], bf16)
        nc.gpsimd.dma_start(wk_sb[:], wkt_v)
        wv_sb = consts.tile([P, KD, QD], bf16)
        nc.gpsimd.dma_start(wv_sb[:], wvt_v)
        bq_sb = consts.tile([P, MQ], f32)
        nc.gpsimd.dma_start(bq_sb[:], bq)
        bk_sb = consts.tile([P, MQ], f32)
        nc.gpsimd.dma_start(bk_sb[:], bk)
        wo_sb = consts.tile([P, MQ, D], bf16)
        nc.gpsimd.dma_start(wo_sb[:], wot_v)
        bv_sb = None  # folded into host combine

        QT = persist.tile([P, MQ, TQ], bf16)
        KgT = persist.tile([P, NP, L], bf16)
        # Vg per (pair j, l-chunk): cols [h0 V | ones | h1 V] (3*DH wide).
        # lhsT for h0 = cols 0:128 (V,ones) -> psum[0:64]=out, [64:128]=denom;
        # lhsT for h1 = cols 64:192 (ones,V) -> psum[0:64]=denom, [64:128]=out.
        Vg = persist.tile([P, NP, LC, 3 * DH], bf16)
        nc.vector.memset(Vg[:, :, :, DH:2 * DH], 1.0)
        outT = persist.tile([P, MQ, TQ], bf16)

        if loop_n > 1:
            with tc.For_i(0, loop_n):
                _emit(nc, tc, mybir, bf16, f32, pA, pB,
                      qt_v, kgt_v, vgt_v, out_v,
                      wq_sb, wk_sb, wv_sb, wo_sb, bq_sb, bk_sb, bv_sb, ident,
                      QT, KgT, Vg, outT,
                      KD, MQ, NP, LC, T2, MT, DH, L, TQ, D, 0)
        else:
            for _rep in range(reps):
                _emit(nc, tc, mybir, bf16, f32, pA, pB,
                      qt_v, kgt_v, vgt_v, out_v,
                      wq_sb, wk_sb, wv_sb, wo_sb, bq_sb, bk_sb, bv_sb, ident,
                      QT, KgT, Vg, outT,
                      KD, MQ, NP, LC, T2, MT, DH, L, TQ, D, _rep)

    nc.compile()
    return nc


def _emit(nc, tc, mybir, bf16, f32, pA, pB,
          qt_v, kgt_v, vgt_v, out_v,
          wq_sb, wk_sb, wv_sb, wo_sb, bq_sb, bk_sb, bv_sb, ident,
          QT, KgT, Vg, outT,
          KD, MQ, NP, LC, T2, MT, DH, L, TQ, D, _rep):
    Exp = mybir.ActivationFunctionType.Exp
    with ExitStack() as ctx:
        # qt stays alive the whole rep: letting its SBUF be reused forced the
        # allocator to gate the kg DMAs on all 128 Q-proj matmuls draining.
        qt_pool = ctx.enter_context(tc.tile_pool(name=f"qtp{_rep}", bufs=1))
        qt_sb = qt_pool.tile([P, KD, TQ], bf16)
        # quarter 0 on SP (PE's first dependency; ACT pays the activation
        # table load first), quarter 1 on ACT in parallel.
        for tq in range(TQ // 512):
            eng = nc.scalar if tq == 1 else nc.sync
            eng.dma_start(
                qt_sb[:, :, tq * 512:(tq + 1) * 512],
                qt_v[:, :, tq * 512:(tq + 1) * 512],
            )
        for t2 in range(T2):
            for m in range(MQ):
                for th in range(2):
                    t0 = t2 * 1024 + th * 512
                    ps = pA.tile([P, 512], f32, tag="A")
                    for kc in range(KD):
                        nc.tensor.matmul(
                            ps[:],
                            wq_sb[:, kc, m * P:(m + 1) * P],
                            qt_sb[:, kc, t0:t0 + 512],
                            start=(kc == 0),
                            stop=(kc == KD - 1),
                        )
                    nc.vector.tensor_scalar_add(
                        QT[:, m, t0:t0 + 512], ps[:], bq_sb[:, m:m + 1]
                    )

        kg_pool = ctx.enter_context(tc.tile_pool(name=f"kg{_rep}", bufs=2))
        vg_pool = ctx.enter_context(tc.tile_pool(name=f"vg{_rep}", bufs=2))
        vt_pool = ctx.enter_context(tc.tile_pool(name=f"vt{_rep}", bufs=2))
        exp_pool = ctx.enter_context(tc.tile_pool(name=f"expp{_rep}", bufs=1))
        norm_pool = ctx.enter_context(tc.tile_pool(name=f"normp{_rep}", bufs=2))
        out_pool = ctx.enter_context(tc.tile_pool(name=f"outp{_rep}", bufs=2))

        def kv_dma(j):
            # Input streams spread over three DMA queues: kg-even on SP,
            # kg-odd on ACT (idle until the first exp), vg on gpsimd SWDGE.
            kg_sb = kg_pool.tile([P, KD, 2, L], bf16, tag="kg")
            kg_eng = nc.scalar if j == 1 else nc.sync
            kg_eng.dma_start(kg_sb[:], kgt_v[:, :, 2 * j:2 * j + 2, :])
            vg_in = vg_pool.tile([P, KD, 2, L], bf16, tag="vg")
            nc.gpsimd.dma_start(vg_in[:], vgt_v[:, :, 2 * j:2 * j + 2, :])
            return kg_sb, vg_in

        def kv_proj(j, kg_sb, vg_in):
            psk = pB.tile([P, 512], f32, tag="B")
            for hh in range(2):
                for kc in range(KD):
                    nc.tensor.matmul(
                        psk[hh * DH:(hh + 1) * DH, :L],
                        wk_sb[:, kc, j * P + hh * DH:j * P + (hh + 1) * DH],
                        kg_sb[:, kc, hh, :],
                        start=(kc == 0),
                        stop=(kc == KD - 1),
                        tile_position=(0, hh * DH),
                    )
            nc.vector.tensor_scalar_add(KgT[:, j, :], psk[:, :L], bk_sb[:, j:j + 1])

            psv = pB.tile([P, 512], f32, tag="B")
            for hh in range(2):
                for kc in range(KD):
                    nc.tensor.matmul(
                        psv[hh * DH:(hh + 1) * DH, :L],
                        wv_sb[:, kc, j * P + hh * DH:j * P + (hh + 1) * DH],
                        vg_in[:, kc, hh, :],
                        start=(kc == 0),
                        stop=(kc == KD - 1),
                        tile_position=(0, hh * DH),
                    )
            vgt_sb = vt_pool.tile([P, L], bf16, tag="vgt")
            nc.vector.tensor_copy(vgt_sb[:], psv[:, :L])
            for l in range(LC):
                pst = pB.tile([P, P], bf16, tag="B")
                nc.tensor.transpose(pst[:], vgt_sb[:, l * P:(l + 1) * P], ident)
                # h0 V -> cols 0:64, h1 V -> cols 128:192 in one strided copy
                dst = Vg[:, j, l, :].rearrange("p (a dh) -> p a dh", a=3)[:, 0::2, :]
                src = pst[:].rearrange("p (a dh) -> p a dh", a=2)
                nc.vector.tensor_copy(dst, src)

        def scores_part(j, t2):
            # all 16 score+exp tiles emit before any PV: each PV block then
            # gates on exps finished 2-3 blocks ago, so the ACT deficit
            # (exp ~9.8us/att vs PE 6.8us) hides under PE's own score work
            expst = {
                (hh, th): exp_pool.tile([P, LC, 512], bf16,
                                        tag=f"e{hh}{th}", name=f"e{hh}{th}")
                for hh in range(2) for th in range(2)
            }
            for hh in range(2):
                hsl = slice(hh * DH, (hh + 1) * DH)
                for th in range(2):
                    t0 = t2 * 1024 + th * 512
                    for l in range(LC):
                        pss = pA.tile([P, 512], f32, tag="A")
                        nc.tensor.matmul(
                            pss[:],
                            KgT[hsl, j, l * P:(l + 1) * P],
                            QT[hsl, j, t0:t0 + 512],
                            start=True,
                            stop=True,
                        )
                        nc.scalar.activation(
                            expst[hh, th][:, l, :], pss[:], Exp, scale=0.125,
                        )
            return expst

        def pv_part(j, t2, expst):
            for hh in range(2):
                hsl = slice(hh * DH, (hh + 1) * DH)
                # h0: out rows 0:64, denom rows 64:128; h1: swapped
                osl = slice(0, DH) if hh == 0 else slice(DH, 2 * DH)
                dsl = slice(DH, 2 * DH) if hh == 0 else slice(0, DH)
                for th in range(2):
                    t0 = t2 * 1024 + th * 512
                    pspv = pB.tile([P, 512], f32, tag="B")
                    for l in range(LC):
                        nc.tensor.matmul(
                            pspv[:],
                            Vg[:, j, l, hh * DH:hh * DH + 2 * DH],
                            expst[hh, th][:, l, :],
                            start=(l == 0),
                            stop=(l == LC - 1),
                        )
                    recip = norm_pool.tile([DH, 512], f32, tag="recip")
                    nc.vector.reciprocal(recip[:], pspv[dsl, :])
                    nc.vector.tensor_mul(
                        outT[hsl, j, t0:t0 + 512], pspv[osl, :], recip[:]
                    )
            # V bias is folded into the host combine: (attn+bv)@WoT =
            # attn@WoT + bv@WoT, and bv@WoT is a constant row added with bo.

        def out_proj(mts, pool=None, ptag="A"):
            # psums in pA (scores pool, idle during out-proj): decouples the
            # out-proj from pB's pspv rotation, whose readers are the DVE
            # normalize chain.
            pool = pool or pA
            for mt in mts:
                ob = out_pool.tile([P, D], bf16, tag="ob")
                for th in range(2):
                    pso = pool.tile([P, 512], f32, tag=ptag, name="pso")
                    for jo in range(MQ):
                        nc.tensor.matmul(
                            pso[:],
                            outT[:, jo, mt * P:(mt + 1) * P],
                            wo_sb[:, jo, th * 512:(th + 1) * 512],
                            start=(jo == 0),
                            stop=(jo == MQ - 1),
                        )
                    nc.vector.tensor_copy(ob[:, th * 512:(th + 1) * 512], pso[:])
                out_eng = nc.gpsimd if mt % 2 == 0 else nc.sync
                out_eng.dma_start(out_v[:, mt, :], ob[:])

        # kv DMAs prefetched one pair ahead (bufs=2) so transfer of pair j+1
        # overlaps compute of pair j on the single SP HWDGE queue.  Software
        # pipeline: kv_proj(j+1)'s exp-free PE work sits between scores(j)
        # and PV(j) so the Activation engine finishes j's exps in time.
        pending = {0: kv_dma(0), 1: kv_dma(1)}
        kv_proj(0, *pending.pop(0))
        for j in range(NP):
            exp_new = scores_part(j, 0)
            if j + 2 < NP:
                pending[j + 2] = kv_dma(j + 2)
            if j + 1 < NP:
                kv_proj(j + 1, *pending.pop(j + 1))
            pv_part(j, 0, exp_new)
        # t2=0 out-proj chunks play the same role for the t2=1 blocks.
        HM = MT // T2 // NP
        for j in range(NP):
            expst = scores_part(j, 1)
            out_proj(range(j * HM, (j + 1) * HM))
            pv_part(j, 1, expst)
        out_proj(range(MT // T2, MT))


_PROG = None


def _get_program():
    global _PROG
    if _PROG is None:
        _PROG = build_program(D=D_F, TQ=TQ_F, L=L_F, NH=NH_LOC, num_devices=NCORES)
    return _PROG


def make_core_inputs(q, k, v, Wq, bq, Wk, bk, Wv, bv, Wo, bo):
    D, L, S, NH = D_F, L_F, STRIDE, NH_LOC
    QD = NH * DH
    MQ = QD // P

    q = np.ascontiguousarray(np.asarray(q, np.float32))
    k = np.ascontiguousarray(np.asarray(k, np.float32))
    v = np.ascontiguousarray(np.asarray(v, np.float32))

    qT = [np.ascontiguousarray(q[b].T).astype(BF16) for b in range(B)]
    kr = [k[b].reshape(L, S, D) for b in range(B)]
    vr = [v[b].reshape(L, S, D) for b in range(B)]

    WqT = np.ascontiguousarray(np.asarray(Wq, np.float32).T).astype(BF16)
    WkT = np.ascontiguousarray(np.asarray(Wk, np.float32).T).astype(BF16)
    WvT = np.ascontiguousarray(np.asarray(Wv, np.float32).T).astype(BF16)
    WoT = np.ascontiguousarray(np.asarray(Wo, np.float32).T).astype(BF16)
    bq = np.asarray(bq, np.float32)
    bk = np.asarray(bk, np.float32)
    bv = np.asarray(bv, np.float32)

    in_maps = []
    for c in range(NCORES):
        b, g = divmod(c, 2)
        gsl = slice(g * QD, (g + 1) * QD)
        hs0 = g * NH
        kgt = np.ascontiguousarray(
            kr[b][:, hs0:hs0 + NH, :].transpose(2, 1, 0)
        ).astype(BF16)
        vgt = np.ascontiguousarray(
            vr[b][:, hs0:hs0 + NH, :].transpose(2, 1, 0)
        ).astype(BF16)
        in_maps.append(
            {
                "qt": qT[b],
                "kgt": kgt,
                "vgt": vgt,
                "wqt": np.ascontiguousarray(WqT[:, gsl]),
                "wkt": np.ascontiguousarray(WkT[:, gsl]),
                "wvt": np.ascontiguousarray(WvT[:, gsl]),
                "wot": np.ascontiguousarray(WoT[gsl, :]),
                "bq": np.ascontiguousarray(bq[gsl].reshape(MQ, P).T),
                "bk": np.ascontiguousarray(bk[gsl].reshape(MQ, P).T),
            }
        )
    return in_maps


def combine_outputs(results, bo, bv, Wo):
    # device partials exclude the V bias: (attn+bv)@Wo.T = attn@Wo.T + bv@Wo.T,
    # so add bv@Wo.T (a constant row) here along with bo.
    bo = np.asarray(bo, np.float32)
    bvWo = np.asarray(bv, np.float32) @ np.asarray(Wo, np.float32).T
    out = np.empty((B, TQ_F, D_F), np.float32)
    for b in range(B):
        out[b] = (
            results[2 * b]["out"].astype(np.float32)
            + results[2 * b + 1]["out"].astype(np.float32)
            + (bo + bvWo)
        )
    return out


def kernel(q, k, v, Wq, bq, Wk, bk, Wv, bv, Wo, bo):
    from concourse.bass_utils import run_bass_kernel_spmd

    nc = _get_program()
    in_maps = make_core_inputs(q, k, v, Wq, bq, Wk, bk, Wv, bv, Wo, bo)
    res = run_bass_kernel_spmd(nc, in_maps, core_ids=list(range(NCORES)))
    return combine_outputs(res.results, bo, bv, Wo)


# revision 63
# speedup vs baseline: 1.1709x; 1.0676x over previous
"""HEPOS multi-head attention on 8 Trainium2 NeuronCores.

Sharding: 8 cores = 4 batches x 2 head-groups (8 heads each).  Since
stride == n_head, each key position feeds exactly one head, so K/V only
ever project through that head's 64 columns (16x less K/V-proj work than
the dense reference).  Per core: Q-proj -> per-head-pair K/V-proj ->
scores+exp -> PV (with an all-ones column block emitting the softmax
denominator for free) -> normalize -> out-proj.  Host sums the two
head-group partials per batch.

Performance structure (sim ~160us/core at 87% PE occupancy, vs ~201us
for the first working version whose HW time was ~615us):
  - software-pipelined attention: scores(j)+exp emit, then exp-free PE
    work (kv-proj of pair j+1 at t2=0, out-proj chunks at t2=1), then
    PV(j) — the Activation engine's exp deficit (9.8us/att vs PE 6.8us)
    hides under PE work instead of stalling it
  - all matmul operands bf16; every psum tile 512-wide (PSUM bank limit),
    rotating through two 4-deep one-bank pools (pA: Q-proj/scores/
    out-proj, pB: KV-proj/PV) so no producer ever waits on a consumer
  - both heads' score blocks are emitted before either PV block, so the
    Activation engine's exp (the attention-phase bottleneck at 1
    elem/cycle, dtype-independent) trails the PE without blocking it
  - qt stays in SBUF for the whole rep: letting its space be reused
    made the allocator gate the kg DMAs on all 128 Q-proj matmuls
  - input DMAs spread across the three DMA queues (SP / ACT / gpsimd
    SWDGE) and kg/vg prefetched one pair ahead: no single-queue
    serialization on the critical path
  - output partials stored bf16 (halves store + host-transfer bytes);
    host upcasts and combines in f32
  - Vg layout per (pair, l-chunk) is [h0 V | ones | h1 V] so one strided
    copy fills both heads and the ones block is shared
  - Q/K biases applied on-device; V bias folded into the host combine
    ((attn+bv)@Wo.T = attn@Wo.T + bv@Wo.T, a constant row); torch
    Linear y = x @ W.T + b matched throughout
"""

import numpy as np
from contextlib import ExitStack

import ml_dtypes

P = 128
BF16 = ml_dtypes.bfloat16

B, TQ_F, TK_F, D_F = 4, 2048, 8192, 1024
H_F, DH, STRIDE = 16, 64, 16
NCORES = 8
NH_LOC = H_F // 2
L_F = TK_F // STRIDE


def build_program(D=1024, TQ=2048, L=512, NH=8, num_devices=8, reps=1, loop_n=1):
    import concourse.bass as bass  # noqa: F401
    import concourse.tile as tile
    from concourse import bacc, mybir
    from concourse.masks import make_identity

    bf16 = mybir.dt.bfloat16
    f32 = mybir.dt.float32

    QD = NH * DH               # 512
    KD = D // P                # 8
    MQ = QD // P               # 4
    NP = NH // 2               # 4 head pairs
    LC = L // P                # 4
    T2 = TQ // 1024            # 2
    MT = TQ // P               # 16
    assert QD % P == 0 and D % P == 0 and L % P == 0 and TQ % 1024 == 0

    nc = bacc.Bacc(
        "TRN2",
        target_bir_lowering=False,
        debug=False,
        enable_asserts=False,
        num_devices=num_devices,
    )

    qt = nc.dram_tensor("qt", [D, TQ], bf16, kind="ExternalInput").ap()
    kgt = nc.dram_tensor("kgt", [D, NH, L], bf16, kind="ExternalInput").ap()
    vgt = nc.dram_tensor("vgt", [D, NH, L], bf16, kind="ExternalInput").ap()
    wqt = nc.dram_tensor("wqt", [D, QD], bf16, kind="ExternalInput").ap()
    wkt = nc.dram_tensor("wkt", [D, QD], bf16, kind="ExternalInput").ap()
    wvt = nc.dram_tensor("wvt", [D, QD], bf16, kind="ExternalInput").ap()
    wot = nc.dram_tensor("wot", [QD, D], bf16, kind="ExternalInput").ap()
    bq = nc.dram_tensor("bq", [P, MQ], f32, kind="ExternalInput").ap()
    bk = nc.dram_tensor("bk", [P, MQ], f32, kind="ExternalInput").ap()
    out = nc.dram_tensor("out", [TQ, D], bf16, kind="ExternalOutput").ap()

    qt_v = qt.rearrange("(kc p) t -> p kc t", p=P)
    kgt_v = kgt.rearrange("(kc p) h l -> p kc h l", p=P)
    vgt_v = vgt.rearrange("(kc p) h l -> p kc h l", p=P)
    wqt_v = wqt.rearrange("(kc p) m -> p kc m", p=P)
    wkt_v = wkt.rearrange("(kc p) m -> p kc m", p=P)
    wvt_v = wvt.rearrange("(kc p) m -> p kc m", p=P)
    wot_v = wot.rearrange("(j p) o -> p j o", p=P)
    out_v = out.rearrange("(mt p) o -> p mt o", p=P)

    Exp = mybir.ActivationFunctionType.Exp

    with tile.TileContext(nc) as tc, ExitStack() as octx:
        consts = octx.enter_context(tc.tile_pool(name="consts", bufs=1))
        persist = octx.enter_context(tc.tile_pool(name="persist", bufs=1))
        # pA: Q-proj + scores psums (512-wide, deep rotation so ACT exp can
        # trail PE without blocking it); pB: KV-proj + PV + out-proj psums.
        pA = octx.enter_context(tc.tile_pool(name="pA", bufs=6, space="PSUM"))
        pB = octx.enter_context(tc.tile_pool(name="pB", bufs=2, space="PSUM"))

        # Weights/biases go on the gpsimd (SWDGE) queue so the latency-critical
        # qt/kg/vg loads own the SP HWDGE queue; wq first (Q-proj needs it).
        ident = consts.tile([P, P], bf16)
        make_identity(nc, ident)
        # first m-chunk of wq on ACT (parallel with qt quarter-0 on SP) so
        # the first Q-proj psum group can start ~3.4us in; rest on gpsimd
        wq_sb = consts.tile([P, KD, QD], bf16)
        nc.scalar.dma_start(wq_sb[:, :, 0:P], wqt_v[:, :, 0:P])
        nc.gpsimd.dma_start(wq_sb[:, :, P:], wqt_v[:, :, P:])
        wk_sb = consts.tile([P, KD, QD], bf16)
        nc.gpsimd.dma_start(wk_sb[:], wkt_v)
        wv_sb = consts.tile([P, KD, QD], bf16)
        nc.gpsimd.dma_start(wv_sb[:], wvt_v)
        bq_sb = consts.tile([P, MQ], f32)
        nc.gpsimd.dma_start(bq_sb[:], bq)
        bk_sb = consts.tile([P, MQ], f32)
        nc.gpsimd.dma_start(bk_sb[:], bk)
        wo_sb = consts.tile([P, MQ, D], bf16)
        nc.gpsimd.dma_start(wo_sb[:], wot_v)
        bv_sb = None  # folded into host combine

        QT = persist.tile([P, MQ, TQ], bf16)
        KgT = persist.tile([P, NP, L], bf16)
        # Vg per (pair j, l-chunk): cols [h0 V | ones | h1 V] (3*DH wide).
        # lhsT for h0 = cols 0:128 (V,ones) -> psum[0:64]=out, [64:128]=denom;
        # lhsT for h1 = cols 64:192 (ones,V) -> psum[0:64]=denom, [64:128]=out.
        Vg = persist.tile([P, NP, LC, 3 * DH], bf16)
        nc.vector.memset(Vg[:, :, :, DH:2 * DH], 1.0)
        outT = persist.tile([P, MQ, TQ], bf16)

        if loop_n > 1:
            with tc.For_i(0, loop_n):
                _emit(nc, tc, mybir, bf16, f32, pA, pB,
                      qt_v, kgt_v, vgt_v, out_v,
                      wq_sb, wk_sb, wv_sb, wo_sb, bq_sb, bk_sb, bv_sb, ident,
                      QT, KgT, Vg, outT,
                      KD, MQ, NP, LC, T2, MT, DH, L, TQ, D, 0)
        else:
            for _rep in range(reps):
                _emit(nc, tc, mybir, bf16, f32, pA, pB,
                      qt_v, kgt_v, vgt_v, out_v,
                      wq_sb, wk_sb, wv_sb, wo_sb, bq_sb, bk_sb, bv_sb, ident,
                      QT, KgT, Vg, outT,
                      KD, MQ, NP, LC, T2, MT, DH, L, TQ, D, _rep)

    nc.compile()
    return nc


def _emit(nc, tc, mybir, bf16, f32, pA, pB,
          qt_v, kgt_v, vgt_v, out_v,
          wq_sb, wk_sb, wv_sb, wo_sb, bq_sb, bk_sb, bv_sb, ident,
          QT, KgT, Vg, outT,
          KD, MQ, NP, LC, T2, MT, DH, L, TQ, D, _rep):
    Exp = mybir.ActivationFunctionType.Exp
    with ExitStack() as ctx:
        # qt stays alive the whole rep: letting its SBUF be reused forced the
        # allocator to gate the kg DMAs on all 128 Q-proj matmuls draining.
        qt_pool = ctx.enter_context(tc.tile_pool(name=f"qtp{_rep}", bufs=1))
        qt_sb = qt_pool.tile([P, KD, TQ], bf16)
        # quarter 0 on SP (PE's first dependency; ACT pays the activation
        # table load first), quarter 1 on ACT in parallel.
        for tq in range(TQ // 512):
            eng = nc.scalar if tq == 1 else nc.sync
            eng.dma_start(
                qt_sb[:, :, tq * 512:(tq + 1) * 512],
                qt_v[:, :, tq * 512:(tq + 1) * 512],
            )
        for t2 in range(T2):
            for m in range(MQ):
                for th in range(2):
                    t0 = t2 * 1024 + th * 512
                    ps = pA.tile([P, 512], f32, tag="A")
                    for kc in range(KD):
                        nc.tensor.matmul(
                            ps[:],
                            wq_sb[:, kc, m * P:(m + 1) * P],
                            qt_sb[:, kc, t0:t0 + 512],
                            start=(kc == 0),
                            stop=(kc == KD - 1),
                        )
                    nc.vector.tensor_scalar_add(
                        QT[:, m, t0:t0 + 512], ps[:], bq_sb[:, m:m + 1]
                    )

        kg_pool = ctx.enter_context(tc.tile_pool(name=f"kg{_rep}", bufs=2))
        vg_pool = ctx.enter_context(tc.tile_pool(name=f"vg{_rep}", bufs=2))
        vt_pool = ctx.enter_context(tc.tile_pool(name=f"vt{_rep}", bufs=2))
        exp_pool = ctx.enter_context(tc.tile_pool(name=f"expp{_rep}", bufs=1))
        norm_pool = ctx.enter_context(tc.tile_pool(name=f"normp{_rep}", bufs=2))
        out_pool = ctx.enter_context(tc.tile_pool(name=f"outp{_rep}", bufs=2))

        def kv_dma(j):
            # Input streams spread over three DMA queues: kg-even on SP,
            # kg-odd on ACT (idle until the first exp), vg on gpsimd SWDGE.
            kg_sb = kg_pool.tile([P, KD, 2, L], bf16, tag="kg")
            kg_eng = nc.scalar if j == 1 else nc.sync
            kg_eng.dma_start(kg_sb[:], kgt_v[:, :, 2 * j:2 * j + 2, :])
            vg_in = vg_pool.tile([P, KD, 2, L], bf16, tag="vg")
            nc.gpsimd.dma_start(vg_in[:], vgt_v[:, :, 2 * j:2 * j + 2, :])
            return kg_sb, vg_in

        def kv_proj(j, kg_sb, vg_in):
            psk = pB.tile([P, 512], f32, tag="B")
            for hh in range(2):
                for kc in range(KD):
                    nc.tensor.matmul(
                        psk[hh * DH:(hh + 1) * DH, :L],
                        wk_sb[:, kc, j * P + hh * DH:j * P + (hh + 1) * DH],
                        kg_sb[:, kc, hh, :],
                        start=(kc == 0),
                        stop=(kc == KD - 1),
                        tile_position=(0, hh * DH),
                    )
            nc.vector.tensor_scalar_add(KgT[:, j, :], psk[:, :L], bk_sb[:, j:j + 1])

            psv = pB.tile([P, 512], f32, tag="B")
            for hh in range(2):
                for kc in range(KD):
                    nc.tensor.matmul(
                        psv[hh * DH:(hh + 1) * DH, :L],
                        wv_sb[:, kc, j * P + hh * DH:j * P + (hh + 1) * DH],
                        vg_in[:, kc, hh, :],
                        start=(kc == 0),
                        stop=(kc == KD - 1),
                        tile_position=(0, hh * DH),
                    )
            vgt_sb = vt_pool.tile([P, L], bf16, tag="vgt")
            nc.vector.tensor_copy(vgt_sb[:], psv[:, :L])
            for l in range(LC):
                pst = pB.tile([P, P], bf16, tag="B")
                nc.tensor.transpose(pst[:], vgt_sb[:, l * P:(l + 1) * P], ident)
                # h0 V -> cols 0:64, h1 V -> cols 128:192 in one strided copy
                dst = Vg[:, j, l, :].rearrange("p (a dh) -> p a dh", a=3)[:, 0::2, :]
                src = pst[:].rearrange("p (a dh) -> p a dh", a=2)
                nc.vector.tensor_copy(dst, src)

        def scores_part(j, t2):
            # all 16 score+exp tiles emit before any PV: each PV block then
            # gates on exps finished 2-3 blocks ago, so the ACT deficit
            # (exp ~9.8us/att vs PE 6.8us) hides under PE's own score work
            expst = {
                (hh, th): exp_pool.tile([P, LC, 512], bf16,
                                        tag=f"e{hh}{th}", name=f"e{hh}{th}")
                for hh in range(2) for th in range(2)
            }
            for hh in range(2):
                hsl = slice(hh * DH, (hh + 1) * DH)
                for th in range(2):
                    t0 = t2 * 1024 + th * 512
                    for l in range(LC):
                        pss = pA.tile([P, 512], f32, tag="A")
                        nc.tensor.matmul(
                            pss[:],
                            KgT[hsl, j, l * P:(l + 1) * P],
                            QT[hsl, j, t0:t0 + 512],
                            start=True,
                            stop=True,
                        )
                        nc.scalar.activation(
                            expst[hh, th][:, l, :], pss[:], Exp, scale=0.125,
                        )
            return expst

        def pv_part(j, t2, expst):
            for hh in range(2):
                hsl = slice(hh * DH, (hh + 1) * DH)
                # h0: out rows 0:64, denom rows 64:128; h1: swapped
                osl = slice(0, DH) if hh == 0 else slice(DH, 2 * DH)
                dsl = slice(DH, 2 * DH) if hh == 0 else slice(0, DH)
                for th in range(2):
                    t0 = t2 * 1024 + th * 512
                    pspv = pB.tile([P, 512], f32, tag="B")
                    for l in range(LC):
                        nc.tensor.matmul(
                            pspv[:],
                            Vg[:, j, l, hh * DH:hh * DH + 2 * DH],
                            expst[hh, th][:, l, :],
                            start=(l == 0),
                            stop=(l == LC - 1),
                        )
                    recip = norm_pool.tile([DH, 512], f32, tag="recip")
                    nc.vector.reciprocal(recip[:], pspv[dsl, :])
                    nc.vector.tensor_mul(
                        outT[hsl, j, t0:t0 + 512], pspv[osl, :], recip[:]
                    )
            # V bias is folded into the host combine: (attn+bv)@WoT =
            # attn@WoT + bv@WoT, and bv@WoT is a constant row added with bo.

        def out_proj(mts, pool=None, ptag="A"):
            # psums in pA (scores pool, idle during out-proj): decouples the
            # out-proj from pB's pspv rotation, whose readers are the DVE
            # normalize chain.
            pool = pool or pA
            for mt in mts:
                ob = out_pool.tile([P, D], bf16, tag="ob")
                for th in range(2):
                    pso = pool.tile([P, 512], f32, tag=ptag, name="pso")
                    for jo in range(MQ):
                        nc.tensor.matmul(
                            pso[:],
                            outT[:, jo, mt * P:(mt + 1) * P],
                            wo_sb[:, jo, th * 512:(th + 1) * 512],
                            start=(jo == 0),
                            stop=(jo == MQ - 1),
                        )
                    nc.vector.tensor_copy(ob[:, th * 512:(th + 1) * 512], pso[:])
                out_eng = nc.gpsimd if mt % 2 == 0 else nc.sync
                out_eng.dma_start(out_v[:, mt, :], ob[:])

        # kv DMAs prefetched one pair ahead (bufs=2) so transfer of pair j+1
        # overlaps compute of pair j on the single SP HWDGE queue.  Software
        # pipeline: kv_proj(j+1)'s exp-free PE work sits between scores(j)
        # and PV(j) so the Activation engine finishes j's exps in time.
        pending = {0: kv_dma(0), 1: kv_dma(1)}
        kv_proj(0, *pending.pop(0))
        for j in range(NP):
            exp_new = scores_part(j, 0)
            if j + 2 < NP:
                pending[j + 2] = kv_dma(j + 2)
            if j + 1 < NP:
                kv_proj(j + 1, *pending.pop(j + 1))
            pv_part(j, 0, exp_new)
        # t2=0 out-proj chunks play the same role for the t2=1 blocks.
        HM = MT // T2 // NP
        for j in range(NP):
            expst = scores_part(j, 1)
            out_proj(range(j * HM, (j + 1) * HM))
            pv_part(j, 1, expst)
        out_proj(range(MT // T2, MT))


_PROG = None


def _get_program():
    global _PROG
    if _PROG is None:
        _PROG = build_program(D=D_F, TQ=TQ_F, L=L_F, NH=NH_LOC, num_devices=NCORES)
    return _PROG


def make_core_inputs(q, k, v, Wq, bq, Wk, bk, Wv, bv, Wo, bo):
    D, L, S, NH = D_F, L_F, STRIDE, NH_LOC
    QD = NH * DH
    MQ = QD // P

    q = np.ascontiguousarray(np.asarray(q, np.float32))
    k = np.ascontiguousarray(np.asarray(k, np.float32))
    v = np.ascontiguousarray(np.asarray(v, np.float32))

    qT = [np.ascontiguousarray(q[b].T).astype(BF16) for b in range(B)]
    kr = [k[b].reshape(L, S, D) for b in range(B)]
    vr = [v[b].reshape(L, S, D) for b in range(B)]

    WqT = np.ascontiguousarray(np.asarray(Wq, np.float32).T).astype(BF16)
    WkT = np.ascontiguousarray(np.asarray(Wk, np.float32).T).astype(BF16)
    WvT = np.ascontiguousarray(np.asarray(Wv, np.float32).T).astype(BF16)
    WoT = np.ascontiguousarray(np.asarray(Wo, np.float32).T).astype(BF16)
    bq = np.asarray(bq, np.float32)
    bk = np.asarray(bk, np.float32)
    bv = np.asarray(bv, np.float32)

    in_maps = []
    for c in range(NCORES):
        b, g = divmod(c, 2)
        gsl = slice(g * QD, (g + 1) * QD)
        hs0 = g * NH
        kgt = np.ascontiguousarray(
            kr[b][:, hs0:hs0 + NH, :].transpose(2, 1, 0)
        ).astype(BF16)
        vgt = np.ascontiguousarray(
            vr[b][:, hs0:hs0 + NH, :].transpose(2, 1, 0)
        ).astype(BF16)
        in_maps.append(
            {
                "qt": qT[b],
                "kgt": kgt,
                "vgt": vgt,
                "wqt": np.ascontiguousarray(WqT[:, gsl]),
                "wkt": np.ascontiguousarray(WkT[:, gsl]),
                "wvt": np.ascontiguousarray(WvT[:, gsl]),
                "wot": np.ascontiguousarray(WoT[gsl, :]),
                "bq": np.ascontiguousarray(bq[gsl].reshape(MQ, P).T),
                "bk": np.ascontiguousarray(bk[gsl].reshape(MQ, P).T),
            }
        )
    return in_maps


def combine_outputs(results, bo, bv, Wo):
    # device partials exclude the V bias: (attn+bv)@Wo.T = attn@Wo.T + bv@Wo.T,
    # so add bv@Wo.T (a constant row) here along with bo.
    bo = np.asarray(bo, np.float32)
    bvWo = np.asarray(bv, np.float32) @ np.asarray(Wo, np.float32).T
    out = np.empty((B, TQ_F, D_F), np.float32)
    for b in range(B):
        out[b] = (
            results[2 * b]["out"].astype(np.float32)
            + results[2 * b + 1]["out"].astype(np.float32)
            + (bo + bvWo)
        )
    return out


def kernel(q, k, v, Wq, bq, Wk, bk, Wv, bv, Wo, bo):
    from concourse.bass_utils import run_bass_kernel_spmd

    nc = _get_program()
    in_maps = make_core_inputs(q, k, v, Wq, bq, Wk, bk, Wv, bv, Wo, bo)
    res = run_bass_kernel_spmd(nc, in_maps, core_ids=list(range(NCORES)))
    return combine_outputs(res.results, bo, bv, Wo)
